# revision 5
# baseline (speedup 1.0000x reference)
"""Trainium2 Bass kernel for nn_ReasonerModel — v3: software-pipelined.

Same math/layouts as v2 (host-preformatted bf16 weights, DMA-xbar transposes,
q-padded backward transposes), plus:
- Per-head-pair interleave: each batch's attention is interleaved with the
  NEXT batch's K^T matmul groups, so softmax/exp/transpose latencies hide
  under K matmuls and the PE never idles inside attention.
- knowT streamed as lo/hi s-halves so the next batch's DMA overlaps the
  current batch's V matmuls.
- V of batch 0 of the NEXT layer fills the LN1 bubble; K of the next layer's
  batch 0 interleaves the last batch's attention; so residual/LN chains are
  fully covered by PE work and the HAM clock never re-throttles.
- kT psum evacuation on DVE (tensor_scalar_add) keeping ACT exp-only during
  attention (no activation-table thrash).
- LN chains batched across the 4 batch rows: one Sqrt for all, bf16 cast
  produced by ACT via scale/bias (no gpsimd cast, no serial cast chain).
"""

import os
import sys

sys.path.insert(0, "/opt/trn_rl_repo")

import numpy as np
import ml_dtypes

import concourse.bass as bass
import concourse.tile as tile
from concourse import mybir
from concourse.bass_utils import run_bass_kernel_spmd
from concourse.vector_clock import ScopedClock

B, SQ, SKV, D, H = 32, 80, 1024, 1024, 16
L = int(os.environ.get("KERNEL_LAYERS", "12"))
HD = D // H          # 64
N_CORES = 8
BL = B // N_CORES    # 4
DT = D // 128        # 8
FT = 4 * D // 128    # 32
EPS = 1e-5

F32 = mybir.dt.float32
BF16 = mybir.dt.bfloat16
FP8 = mybir.dt.float8e4
AF = mybir.ActivationFunctionType
ALU = mybir.AluOpType
KV8 = os.environ.get("KV8", "1") == "1"   # fp8 e4m3 + DoubleRow KV matmuls
FP8_SCALE = 64.0                          # host pre-scale of know/wk/wv
FP8_INV2 = 1.0 / (FP8_SCALE * FP8_SCALE)  # descale: K at exp, V at proj evac
KVDT = FP8 if KV8 else BF16


class PatchedTC(tile.TileContext):
    """This container's walrus accepts at most ONE sem wait per instruction;
    Tile may attach several. Peel extras onto preceding same-engine no-ops."""

    def _commit_instruction(self, inst, lazy_reg_writes: bool = True):
        si = getattr(inst, "sync_info", None)
        if (
            si is not None
            and si.on_wait
            and len(si.on_wait) > 1
            and inst.engine != mybir.EngineType.Unassigned
        ):
            waits = list(si.on_wait)
            si.on_wait = [waits[-1]]
            for j, w in enumerate(waits[:-1]):
                nop = mybir.InstNoOp(
                    name=f"{inst.name}-sw{j}",
                    sync_info=mybir.SyncInfo(on_wait=[w], on_update=[]),
                    bass_nofuse=True,
                    engine=inst.engine,
                )
                super()._commit_instruction(nop, lazy_reg_writes=False)
        return super()._commit_instruction(inst, lazy_reg_writes)

    def _drain_and_barrier(self, tick_clock, wait_clock):
        drain_inst = self.nc.sync.drain()
        wait_clock.add_sem_waits(
            drain_inst.ins, ScopedClock({None: tick_clock.global_clock})
        )
        si = drain_inst.ins.sync_info
        if si is not None and si.on_wait and len(si.on_wait) > 1:
            waits = list(si.on_wait)
            si.on_wait = waits[:1]
            for w in waits[1:]:
                extra = self.nc.sync.drain()
                nsi = extra.ins.sync_info
                if nsi is None:
                    extra.ins.sync_info = mybir.SyncInfo(on_wait=[w], on_update=[])
                else:
                    nsi.on_wait = [w]
        self.nc.all_engine_barrier()
        assert self.sems is not None
        popped = self.nc._tile_sem_poison_stack.pop()
        assert popped is self._sem_poison
        self.nc.clear_and_free_semaphores(list(self.sems.allocated().values()))
        self.nc.all_engine_barrier()


def build_nc(affine: bool, biasless: bool):
    try:
        from concourse import tile_utils

        tile_utils.max_sbuf_usage = 208 * 1024
    except Exception:
        pass

    nc = bass.Bass("TRN2", target_bir_lowering=False, debug=False,
                   num_devices=N_CORES)

    x_in = nc.dram_tensor("xp", [BL, SQ, D], F32, kind="ExternalInput")
    hT0_in = nc.dram_tensor("hT0", [128, DT, BL, SQ], BF16, kind="ExternalInput")
    knowT_in = nc.dram_tensor("knowT", [BL, 128, DT, SKV], KVDT,
                              kind="ExternalInput")
    wk_d = nc.dram_tensor("wk", [L, 128, DT, D], KVDT, kind="ExternalInput")
    wv_d = nc.dram_tensor("wv", [L, 128, DT, D], KVDT, kind="ExternalInput")
    wp_d = nc.dram_tensor("wp", [L, 4, 128, DT, 256], BF16, kind="ExternalInput")
    wf_d = nc.dram_tensor("wf", [L, 16, 128, DT, 256], BF16,
                          kind="ExternalInput")
    wm_d = nc.dram_tensor("wm", [L, DT, 2, 128, 16, 128], BF16,
                          kind="ExternalInput")
    # ball cols: [0:8]=bk  [8:16]=bp  [16:24]=bm  [24:56]=bf
    ball_d = nc.dram_tensor("ball", [L, 128, 56], F32, kind="ExternalInput")
    NV = 5 if affine else 1
    vecs_d = nc.dram_tensor("vecs", [L, 128, NV, D], BF16, kind="ExternalInput")
    out_ext = nc.dram_tensor("out", [BL, SQ, D], F32, kind="ExternalOutput")

    with PatchedTC(nc) as tc:
        import contextlib

        ctx = contextlib.ExitStack()
        with ctx:
            P = lambda **kw: ctx.enter_context(tc.tile_pool(**kw))
            singles = P(name="singles", bufs=1)
            wkv_pool = P(name="wkv", bufs=1)
            wch_pool = P(name="wch", bufs=3)      # wp / wf / wm chunks
            knb_pool = P(name="knb", bufs=2)      # lo / hi tags
            kT_pool = P(name="kTp", bufs=2)
            vb_pool = P(name="vbp", bufs=1)
            xT_pool = P(name="xT", bufs=2)        # hT / pT rotation
            aT_pool = P(name="aT", bufs=1)
            gT_pool = P(name="gT", bufs=1)
            oT_pool = P(name="oT", bufs=1)        # aoT_b / moT_b per-b tiles
            w_pool = P(name="wsm", bufs=3)        # softmax weights (natural)
            wT_pool = P(name="wT", bufs=6)        # softmax weights (transposed)
            nat_pool = P(name="nat", bufs=2)      # bf16 bounce tiles
            vec_pool = P(name="vec", bufs=2)
            sb_pool = P(name="sb", bufs=2)
            st_pool = P(name="st", bufs=6)
            ps_kv = P(name="pskv", bufs=3, space="PSUM")
            ps_att = P(name="psatt", bufs=2, space="PSUM")
            ps_av = P(name="psav", bufs=1, space="PSUM")

            eps_t = singles.tile([128, 1], F32)
            nc.vector.memset(eps_t, EPS)

            xs = [
                singles.tile([SQ, D], F32, tag=f"x{b}", name=f"x{b}")
                for b in range(BL)
            ]

            # ---------- helpers ----------
            def dma_weights(l):
                wk_sb = wkv_pool.tile([128, DT, D], KVDT, tag="wk",
                                      name=f"wk{l}")
                nc.sync.dma_start(out=wk_sb, in_=wk_d[l])
                wv_sb = wkv_pool.tile([128, DT, D], KVDT, tag="wv",
                                      name=f"wv{l}")
                nc.sync.dma_start(out=wv_sb, in_=wv_d[l])
                vecs_sb = vec_pool.tile([128, NV, D], BF16, tag="vecs",
                                        name=f"vecs{l}")
                nc.gpsimd.dma_start(out=vecs_sb, in_=vecs_d[l])
                ball_sb = sb_pool.tile([128, 56], F32, tag="ball",
                                       name=f"ball{l}")
                nc.gpsimd.dma_start(out=ball_sb, in_=ball_d[l])
                return dict(wk=wk_sb, wv=wv_sb, vecs=vecs_sb, ball=ball_sb)

            knb_tiles = {}

            def load_knb_half(u, half):
                b = u % BL
                t = knb_pool.tile([128, DT, 512], KVDT, tag=f"knb{half}",
                                  name=f"knb{u}h{half}")
                nc.gpsimd.dma_start(
                    out=t, in_=knowT_in[b][:, :, half * 512:(half + 1) * 512])
                knb_tiles.setdefault(u, {})["lo" if half == 0 else "hi"] = t

            def prefetch_hook(u_next, total_units):
                def hook(st):
                    if u_next >= total_units:
                        return
                    if st == 3:
                        load_knb_half(u_next, 0)
                    elif st == 7:
                        load_knb_half(u_next, 1)
                return hook

            def emit_K_half(W, kn, kTb, nt, sc):
                """8 accumulating MMs for K^T tile (nt, s-half sc) + DVE evac."""
                pk = ps_kv.tile([128, 512], F32, tag="kv", name=f"pk{nt}{sc}")
                if KV8:
                    for k2 in range(DT // 2):
                        nc.tensor.matmul(
                            pk,
                            lhsT=W["wk"][:, 2 * k2:2 * k2 + 2,
                                         nt * 128:(nt + 1) * 128],
                            rhs=kn[:, 2 * k2:2 * k2 + 2, :],
                            start=(k2 == 0), stop=(k2 == DT // 2 - 1),
                            perf_mode=mybir.MatmulPerfMode.DoubleRow)
                else:
                    for kt in range(DT):
                        nc.tensor.matmul(
                            pk,
                            lhsT=W["wk"][:, kt, nt * 128:(nt + 1) * 128],
                            rhs=kn[:, kt, :],
                            start=(kt == 0), stop=(kt == DT - 1))
                if biasless:
                    nc.vector.tensor_copy(
                        out=kTb[:, nt, sc * 512:(sc + 1) * 512], in_=pk)
                else:
                    nc.vector.tensor_scalar_add(
                        kTb[:, nt, sc * 512:(sc + 1) * 512], pk,
                        W["ball"][:, nt:nt + 1])

            def emit_V(W, knlo, knhi, vb, after_st=None):
                """All 8 st-groups of V; call after_st(st) hook post-group."""
                for st in range(DT):
                    kn = knlo if st < 4 else knhi
                    co = (st % 4) * 128
                    pv = [ps_kv.tile([128, 512], F32, tag="kv",
                                     name=f"pv{st}{nh}") for nh in range(2)]
                    if KV8:
                        for k2 in range(DT // 2):
                            for nh in range(2):
                                nc.tensor.matmul(
                                    pv[nh],
                                    lhsT=kn[:, 2 * k2:2 * k2 + 2, co:co + 128],
                                    rhs=W["wv"][:, 2 * k2:2 * k2 + 2,
                                                nh * 512:(nh + 1) * 512],
                                    start=(k2 == 0),
                                    stop=(k2 == DT // 2 - 1),
                                    perf_mode=mybir.MatmulPerfMode.DoubleRow)
                    else:
                        for kt in range(DT):
                            for nh in range(2):
                                nc.tensor.matmul(
                                    pv[nh],
                                    lhsT=kn[:, kt, co:co + 128],
                                    rhs=W["wv"][:, kt, nh * 512:(nh + 1) * 512],
                                    start=(kt == 0), stop=(kt == DT - 1))
                    for nh in range(2):
                        if biasless:
                            nc.vector.tensor_copy(
                                out=vb[:, st, nh * 512:(nh + 1) * 512],
                                in_=pv[nh])
                        else:
                            nc.vector.tensor_tensor(
                                vb[:, st, nh * 512:(nh + 1) * 512], pv[nh],
                                W["vecs"][:, 0, nh * 512:(nh + 1) * 512],
                                ALU.add)
                    if after_st is not None:
                        after_st(st)

            def emit_scores_head(hT, kTb, b, h):
                """scores + exp + renorm + xbar transpose for one head."""
                hp, hs = h // 2, h % 2
                po = hs * 64
                ps = ps_att.tile([SQ, 2, 512], F32, tag="att", name=f"sc{h}")
                for sc in range(2):
                    nc.tensor.matmul(
                        ps[:, sc, :],
                        lhsT=hT[po:po + 64, hp, b, :],
                        rhs=kTb[po:po + 64, hp, sc * 512:(sc + 1) * 512],
                        start=True, stop=True)
                sume = st_pool.tile([SQ, 1], F32, tag="sume", name="sume")
                w_sb = w_pool.tile([SQ, SKV], BF16, tag="w", name="w_sb")
                esc = (FP8_INV2 if KV8 else 1.0) / np.sqrt(HD)
                nc.scalar.activation(
                    out=w_sb, in_=ps.rearrange("p a s -> p (a s)"),
                    func=AF.Exp, scale=esc, accum_out=sume)
                rec = st_pool.tile([SQ, 1], F32, tag="rec", name="rec")
                nc.vector.reciprocal(rec, sume)
                nc.vector.tensor_scalar_mul(w_sb, w_sb, rec)
                wTt = wT_pool.tile([128, DT, SQ], BF16, tag="wT",
                                   name=f"wT{h}")
                ring = nc.sync if h % 2 == 0 else nc.scalar
                ring.dma_start(out=wTt, in_=w_sb, transpose=True)
                return wTt

            def emit_AV(vb, aT, b, hp, wT_pair):
                pav = ps_av.tile([128, 512], F32, tag="av", name=f"pav{hp}")
                for hs2 in range(2):
                    h2 = 2 * hp + hs2
                    tp = (0, 64) if hs2 == 1 else None
                    for st in range(DT):
                        nc.tensor.matmul(
                            pav[hs2 * 64:(hs2 + 1) * 64, :SQ],
                            lhsT=vb[:, st, h2 * 64:(h2 + 1) * 64],
                            rhs=wT_pair[hs2][:, st, :],
                            start=(st == 0), stop=(st == DT - 1),
                            tile_position=tp)
                nc.vector.tensor_copy(out=aT[:, hp, b, :], in_=pav[:, :SQ])

            def attn_block(hT, kTb, vb, aT, b, nxt):
                """Attention of batch b, interleaved with next KV's K groups.

                nxt: None or dict(W, knlo, knhi, kTb) for the next (layer,
                batch) whose K^T matmuls provide latency cover."""
                wT_prev = None
                nts = list(nxt.get("nts", range(DT))) if nxt is not None else []
                for hp in range(DT):
                    knt = nts[hp] if hp < len(nts) else None
                    if knt is not None:
                        emit_K_half(nxt["W"], nxt["knlo"], nxt["kTb"], knt, 0)
                    wT0 = emit_scores_head(hT, kTb, b, 2 * hp)
                    if knt is not None:
                        emit_K_half(nxt["W"], nxt["knhi"], nxt["kTb"], knt, 1)
                    wT1 = emit_scores_head(hT, kTb, b, 2 * hp + 1)
                    if wT_prev is not None:
                        emit_AV(vb, aT, b, hp - 1, wT_prev)
                    wT_prev = (wT0, wT1)
                emit_AV(vb, aT, b, DT - 1, wT_prev)

            def T_to_resid(srcT_b, x_b):
                natp = nat_pool.tile([128, DT, 128], BF16, tag="natp", name="natp")
                nc.sync.dma_start(
                    out=natp, in_=srcT_b.rearrange("p t q -> p (t q)"),
                    transpose=True)
                nc.vector.tensor_add(
                    x_b, x_b, natp[:SQ].rearrange("p t c -> p (t c)"))

            def ln_make(srcTbs, vecs_sb, gi, dstT, out_dma):
                """Granular residual+LN phase: list of 8 piece-groups, one per
                V st-group hook slot (or run sequentially)."""
                st_ctx = {}

                def resid(b):
                    T_to_resid(srcTbs[b], xs[b])

                def stats(b):
                    if "mvall" not in st_ctx:
                        st_ctx["mvall"] = st_pool.tile([SQ, BL, 2], F32,
                                                       tag="mvall", name="mvall")
                    stt = st_pool.tile([SQ, 2, 6], F32, tag="bnst", name="stt")
                    for c in range(2):
                        nc.vector.bn_stats(stt[:, c, :],
                                           xs[b][:, c * 512:(c + 1) * 512])
                    nc.vector.bn_aggr(st_ctx["mvall"][:, b, :], stt)

                def sqrtrecip():
                    mvall = st_ctx["mvall"]
                    stdall = st_pool.tile([SQ, BL], F32, tag="stdall", name="stdall")
                    nc.scalar.activation(stdall, mvall[:, :, 1], AF.Sqrt,
                                         bias=eps_t[:SQ])
                    nc.vector.reciprocal(stdall, stdall)
                    st_ctx["stdall"] = stdall
                    if not affine and dstT is not None:
                        st_ctx["nmr"] = st_pool.tile([SQ, BL], F32, tag="nmr", name="nmr")

                def tail(b):
                    mvall, stdall = st_ctx["mvall"], st_ctx["stdall"]
                    if not affine:
                        if dstT is not None:
                            nmr = st_ctx["nmr"]
                            nc.vector.tensor_scalar(
                                nmr[:, b:b + 1], mvall[:, b, 0:1],
                                stdall[:, b:b + 1], -1.0,
                                op0=ALU.mult, op1=ALU.mult)
                            natb = nat_pool.tile([SQ, D], BF16, tag="natb", name="natb")
                            nc.scalar.activation(
                                natb, xs[b], AF.Identity,
                                bias=nmr[:, b:b + 1],
                                scale=stdall[:, b:b + 1])
                            nc.scalar.dma_start(out=dstT[:, :, b, :],
                                                in_=natb, transpose=True)
                        nc.vector.tensor_scalar(
                            xs[b], xs[b], mvall[:, b, 0:1],
                            stdall[:, b:b + 1],
                            op0=ALU.subtract, op1=ALU.mult)
                    else:
                        nc.vector.tensor_scalar(
                            xs[b], xs[b], mvall[:, b, 0:1],
                            stdall[:, b:b + 1],
                            op0=ALU.subtract, op1=ALU.mult)
                        nc.gpsimd.tensor_tensor(
                            xs[b], xs[b], vecs_sb[:SQ, gi, :], ALU.mult)
                        nc.gpsimd.tensor_tensor(
                            xs[b], xs[b], vecs_sb[:SQ, gi + 1, :], ALU.add)
                        if dstT is not None:
                            natb = nat_pool.tile([SQ, D], BF16, tag="natb", name="natb")
                            nc.scalar.copy(out=natb, in_=xs[b])
                            nc.scalar.dma_start(out=dstT[:, :, b, :],
                                                in_=natb, transpose=True)
                    if out_dma:
                        nc.sync.dma_start(out=out_ext[b], in_=xs[b])

                return [
                    [lambda: resid(0)],
                    [lambda: resid(1), lambda: stats(0)],
                    [lambda: resid(2), lambda: stats(1)],
                    [lambda: resid(3), lambda: stats(2)],
                    [lambda: stats(3)],
                    [sqrtrecip],
                    [lambda: tail(0), lambda: tail(1)],
                    [lambda: tail(2), lambda: tail(3)],
                ]

            def ln_phase(srcTbs, vecs_sb, gi, dstT, out_dma):
                for group in ln_make(srcTbs, vecs_sb, gi, dstT, out_dma):
                    for p in group:
                        p()

            # ================= prologue =================
            hT = xT_pool.tile([128, DT, BL, SQ], BF16, tag="xT", name="hT0")
            nc.sync.dma_start(out=hT, in_=hT0_in[:, :, :, :])
            for b in range(BL):
                nc.gpsimd.dma_start(out=xs[b], in_=x_in[b])

            TOTAL_U = L * BL
            W = dma_weights(0)
            # preload knb for units 0 and 1 (double-buffered lo/hi tags)
            load_knb_half(0, 0)
            load_knb_half(0, 1)
            load_knb_half(1, 0)
            load_knb_half(1, 1)
            kTb = kT_pool.tile([128, DT, SKV], BF16, tag="kT", name="kT00")
            for nt in range(DT):
                emit_K_half(W, knb_tiles[0]["lo"], kTb, nt, 0)
                emit_K_half(W, knb_tiles[0]["hi"], kTb, nt, 1)
            vb = vb_pool.tile([128, DT, D], BF16, tag="v", name="vb00")
            # V of unit 0 prefetches unit 2
            emit_V(W, knb_tiles[0]["lo"], knb_tiles[0]["hi"], vb,
                   after_st=prefetch_hook(2, TOTAL_U))
            del knb_tiles[0]

            # ================= layers =================
            carry = None
            for l in range(L):
                aT = aT_pool.tile([128, DT, BL, SQ], BF16, tag="aT",
                                  name=f"aT{l}")
                Wn = None
                for b in range(BL):
                    u = l * BL + b          # current unit
                    un = u + 1              # unit whose K interleaves attn
                    if un >= TOTAL_U:
                        nxt = None
                    elif b == 0 and carry is not None:
                        # K of this unit was partially pre-emitted in the
                        # previous layer's LN1 slot; finish nts 4..7 here.
                        nxt = dict(W=W, knlo=knb_tiles[un]["lo"],
                                   knhi=knb_tiles[un]["hi"],
                                   kTb=carry, nts=[4, 5, 6, 7])
                    else:
                        if un % BL == 0:
                            Wn = dma_weights(l + 1)
                        Wx = Wn if un % BL == 0 else W
                        kTn = kT_pool.tile([128, DT, SKV], BF16, tag="kT",
                                           name=f"kT{un}")
                        nxt = dict(W=Wx, knlo=knb_tiles[un]["lo"],
                                   knhi=knb_tiles[un]["hi"], kTb=kTn)

                    attn_block(hT, kTb, vb, aT, b, nxt)

                    if nxt is not None:
                        kTb = nxt["kTb"]
                        knlo, knhi = (knb_tiles[un]["lo"],
                                      knb_tiles[un]["hi"])
                        del knb_tiles[un]
                        hook = prefetch_hook(un + 2, TOTAL_U)
                        if b < BL - 1:
                            # V of unit un now; prefetch knb of un+2
                            vb = vb_pool.tile([128, DT, D], BF16, tag="v",
                                              name=f"vb{un}")
                            emit_V(W, knlo, knhi, vb, after_st=hook)

                # ---- attention out-projection (out^T, q-padded) ----
                aoTb = [oT_pool.tile([128, DT, 128], BF16, tag=f"o{b}",
                                     name=f"ao{b}") for b in range(BL)]
                for c in range(4):
                    wpc = wch_pool.tile([128, DT, 256], BF16, tag="wp",
                                        name=f"wp{c}")
                    nc.sync.dma_start(out=wpc, in_=wp_d[l, c])
                    for ntl in range(2):
                        nt = c * 2 + ntl
                        pp = ps_kv.tile([128, 512], F32, tag="kv",
                                        name=f"pp{nt}")
                        for kt in range(DT):
                            nc.tensor.matmul(
                                pp[:, :BL * SQ],
                                lhsT=wpc[:, kt, ntl * 128:(ntl + 1) * 128],
                                rhs=aT[:, kt, :, :].rearrange(
                                    "p b q -> p (b q)"),
                                start=(kt == 0), stop=(kt == DT - 1))
                        for b in range(BL):
                            nc.scalar.activation(
                                out=aoTb[b][:, nt, :SQ],
                                in_=pp[:, b * SQ:(b + 1) * SQ],
                                func=AF.Identity,
                                scale=FP8_INV2 if KV8 else 1.0,
                                bias=W["ball"][:, 8 + nt:9 + nt])

                # ---- residual + LN1 + pT, interleaved with V(l+1,0) ----
                pT = xT_pool.tile([128, DT, BL, SQ], BF16, tag="xT",
                                  name=f"pT{l}")
                if l < L - 1:
                    sched = ln_make(aoTb, W["vecs"], 1, pT, False)
                    knb_hook = hook

                    def combined(st, _s=sched, _k=knb_hook):
                        _k(st)
                        for p in _s[st]:
                            p()

                    vb = vb_pool.tile([128, DT, D], BF16, tag="v",
                                      name=f"vb{l + 1}0")
                    emit_V(Wn, knlo, knhi, vb, after_st=combined)
                    # pre-emit first half of K of unit (l+1, 1) to cover the
                    # LN1 tail before fc starts
                    un2 = (l + 1) * BL + 1
                    carry = None
                    if un2 < TOTAL_U:
                        carry = kT_pool.tile([128, DT, SKV], BF16, tag="kT",
                                             name=f"kT{un2}")
                        for nt in range(4):
                            emit_K_half(Wn, knb_tiles[un2]["lo"], carry,
                                        nt, 0)
                            emit_K_half(Wn, knb_tiles[un2]["hi"], carry,
                                        nt, 1)
                else:
                    ln_phase(aoTb, W["vecs"], 1, pT, False)

                # ---- ffn in (out^T) + gelu ----
                gT = gT_pool.tile([128, FT, BL, SQ], BF16, tag="gT",
                                  name=f"gT{l}")
                for c in range(16):
                    wfc = wch_pool.tile([128, DT, 256], BF16, tag="wf",
                                        name=f"wf{c}")
                    nc.sync.dma_start(out=wfc, in_=wf_d[l, c])
                    for ntl in range(2):
                        nt = c * 2 + ntl
                        pf = ps_kv.tile([128, 512], F32, tag="kv",
                                        name=f"pf{nt}")
                        for kt in range(DT):
                            nc.tensor.matmul(
                                pf[:, :BL * SQ],
                                lhsT=wfc[:, kt, ntl * 128:(ntl + 1) * 128],
                                rhs=pT[:, kt, :, :].rearrange(
                                    "p b q -> p (b q)"),
                                start=(kt == 0), stop=(kt == DT - 1))
                        nc.scalar.activation(
                            out=gT[:, nt, :, :].rearrange("p b q -> p (b q)"),
                            in_=pf[:, :BL * SQ], func=AF.Gelu_apprx_tanh,
                            bias=W["ball"][:, 24 + nt:25 + nt])

                # ---- ffn out (out^T, q-padded) ----
                moTb = [oT_pool.tile([128, DT, 128], BF16, tag=f"o{b}",
                                     name=f"mo{b}") for b in range(BL)]
                for nt in range(DT):
                    pm = ps_kv.tile([128, 512], F32, tag="kv", name=f"pm{nt}")
                    for half in range(2):
                        wmc = wch_pool.tile([128, 16, 128], BF16, tag="wm",
                                            name=f"wm{nt}{half}")
                        nc.sync.dma_start(out=wmc, in_=wm_d[l, nt, half])
                        for ktl in range(16):
                            kt = half * 16 + ktl
                            nc.tensor.matmul(
                                pm[:, :BL * SQ],
                                lhsT=wmc[:, ktl, :],
                                rhs=gT[:, kt, :, :].rearrange(
                                    "p b q -> p (b q)"),
                                start=(kt == 0), stop=(kt == FT - 1))
                    for b in range(BL):
                        nc.scalar.activation(
                            out=moTb[b][:, nt, :SQ],
                            in_=pm[:, b * SQ:(b + 1) * SQ],
                            func=AF.Identity,
                            bias=W["ball"][:, 16 + nt:17 + nt])

                # ---- residual + LN2 + next hT / output ----
                if l < L - 1:
                    hT = xT_pool.tile([128, DT, BL, SQ], BF16, tag="xT",
                                      name=f"hT{l + 1}")
                    ln_phase(moTb, W["vecs"], 3, hT, False)
                    W = Wn
                else:
                    ln_phase(moTb, W["vecs"], 3, None, True)

    return nc


_CACHE = {}


def _bf(x):
    return np.ascontiguousarray(x.astype(ml_dtypes.bfloat16))


def _f8(x):
    return np.ascontiguousarray(
        (x * FP8_SCALE).astype(ml_dtypes.float8_e4m3))


def _prep(inputs, affine):
    f = lambda k: np.asarray(inputs[k], dtype=np.float32)
    Wa, Wp, Wf, Wm = (f("W_attn")[:L], f("W_proj_attn")[:L], f("W_fc")[:L],
                      f("W_proj_mlp")[:L])
    ba_, bp_, bf_, bm_ = (f("b_attn")[:L], f("b_proj_attn")[:L], f("b_fc")[:L],
                          f("b_proj_mlp")[:L])
    g1, b1, g2, b2 = (f("ln1_g")[:L], f("ln1_b")[:L], f("ln2_g")[:L],
                      f("ln2_b")[:L])

    def wlay(w):  # [L, Din, C] -> [L, 128, Din/128, C] bf16
        Lc, Din, C = w.shape
        return _bf(w.reshape(Lc, Din // 128, 128, C).transpose(0, 2, 1, 3))

    def wchunk(w, nch):  # [L, Din, C] -> [L, nch, 128, Din/128, C/nch]
        Lc, Din, C = w.shape
        r = w.reshape(Lc, Din // 128, 128, nch, C // nch)
        return _bf(r.transpose(0, 3, 2, 1, 4))

    # wm: [L, 4096, 1024] -> [L, 8(nt), 2(half), 128, 16(ktl), 128]
    wm_r = Wm.reshape(L, 2, 16, 128, 8, 128)
    wm_l = _bf(wm_r.transpose(0, 4, 1, 3, 2, 5))

    def wlay8(w):
        Lc, Din, C = w.shape
        return _f8(w.reshape(Lc, Din // 128, 128, C).transpose(0, 2, 1, 3))

    wcast = wlay8 if KV8 else wlay
    shared = {
        "wk": wcast(Wa[:, :, D:2 * D]),
        "wv": wcast(Wa[:, :, 2 * D:3 * D]),
        "wp": wchunk(Wp, 4),
        "wf": wchunk(Wf, 16),
        "wm": wm_l,
    }
    ball = np.zeros((L, 128, 56), np.float32)
    ball[:, :, 0:8] = ba_[:, D:2 * D].reshape(L, 8, 128).transpose(0, 2, 1)
    ball[:, :, 8:16] = bp_.reshape(L, 8, 128).transpose(0, 2, 1)
    ball[:, :, 16:24] = bm_.reshape(L, 8, 128).transpose(0, 2, 1)
    ball[:, :, 24:56] = bf_.reshape(L, 32, 128).transpose(0, 2, 1)
    shared["ball"] = np.ascontiguousarray(ball)
    if affine:
        vecs = np.stack([ba_[:, 2 * D:], g1, b1, g2, b2], axis=1)
    else:
        vecs = ba_[:, None, 2 * D:]
    shared["vecs"] = _bf(np.broadcast_to(
        vecs[:, None], (L, 128) + vecs.shape[1:]))

    xp = f("input_ids") + f("pos_embed")[None]
    know = f("input_ids_know")
    xp_b = xp.astype(ml_dtypes.bfloat16)

    in_maps = []
    for i in range(N_CORES):
        m = dict(shared)
        m["xp"] = np.ascontiguousarray(xp[i * BL:(i + 1) * BL])
        hb = xp_b[i * BL:(i + 1) * BL]
        m["hT0"] = np.ascontiguousarray(
            hb.transpose(2, 0, 1).reshape(DT, 128, BL, SQ).transpose(1, 0, 2, 3))
        kn = know[i * BL:(i + 1) * BL]
        knt = kn.transpose(0, 2, 1).reshape(BL, DT, 128, SKV).transpose(0, 2, 1, 3)
        m["knowT"] = _f8(knt) if KV8 else _bf(knt)
        in_maps.append(m)
    return in_maps


def kernel(**inputs):
    affine = not (
        np.all(np.asarray(inputs["ln1_g"])[:L] == 1.0)
        and np.all(np.asarray(inputs["ln1_b"])[:L] == 0.0)
        and np.all(np.asarray(inputs["ln2_g"])[:L] == 1.0)
        and np.all(np.asarray(inputs["ln2_b"])[:L] == 0.0)
    )
    biasless = (
        np.all(np.asarray(inputs["b_attn"])[:L] == 0.0)
        and np.all(np.asarray(inputs["b_proj_attn"])[:L] == 0.0)
        and np.all(np.asarray(inputs["b_fc"])[:L] == 0.0)
        and np.all(np.asarray(inputs["b_proj_mlp"])[:L] == 0.0)
    )
    key = ("nc", affine, biasless)
    if key not in _CACHE:
        _CACHE[key] = build_nc(affine, biasless)
        _CACHE["nc"] = _CACHE[key]
    nc = _CACHE[key]
    in_maps = _prep(inputs, affine)
    _CACHE["last_in_maps"] = in_maps
    res = run_bass_kernel_spmd(nc, in_maps, list(range(N_CORES)))
    out = np.concatenate([res.results[i]["out"] for i in range(N_CORES)], axis=0)
    return out.astype(np.float32)
